# revision 7
# baseline (speedup 1.0000x reference)
"""Trainium2 kernel for nn_LoRALinear (moe_routing).

Math: reference computes out = x @ W.T + einsum('bri,bro->bo', a, b) with
a = A_table[dom].reshape(B,R,IN), b = B_table[dom].reshape(B,R,OUT).
The einsum contracts i over `a` alone, so the LoRA term collapses to a
per-domain table:
    L[d, o] = sum_r (sum_i A_table[d].reshape(R,IN)[r,i]) * B_table[d].reshape(R,OUT)[r,o]
    out = x @ W.T + L[domain_id]

Device work per core (data-parallel over batch, 2048 rows/core):
    out_tile[mt] = x[mt] @ W.T + Lg[mt]
where Lg = L[domain_id] is gathered on the host (a 64x1024 table lookup)
and streamed alongside x. The dense matmul runs as 16 m-tiles x 8 k-chunks
x 2 n-halves of [128x128] @ [128x512] bf16 MMs with the x block stationary
(LDWEIGHTS overlaps in-flight MMs via the background weight buffer, so the
PE streams at the 512-cycle/MM peak). The LoRA add rides the PSUM->SBUF
drain as a DVE tensor_add, so no partial-row-group matmuls are needed.

Outputs are written bf16 (host upcasts) to halve the store traffic; input
loads go on the sync HWDGE queue and stores on the scalar queue so they
don't head-of-line block each other.
"""

import functools

import numpy as np

import concourse.mybir as mybir
import concourse.tile as tile
from concourse import bacc, bass_utils

B, D, R, ND = 16384, 1024, 8, 64
N_CORES = 8
BS = B // N_CORES            # 2048 batch rows per core
NK = 8                       # k chunks of 128
NMT = BS // 128              # 16 m-tiles per core
MTW = 2 * D                  # xaug cols per m-tile: 1024 x-chunks + 1024 Lg

# m-tiles per DMA block: small blocks first so compute starts early
X_PLAN = [[0], [1], [2, 3], [4, 5, 6, 7], [8, 9, 10, 11], [12, 13, 14, 15]]


@functools.lru_cache(maxsize=1)
def _build():
    nc = bacc.Bacc(None, target_bir_lowering=False, debug=False)
    bf16 = mybir.dt.bfloat16
    f32 = mybir.dt.float32
    xa = nc.dram_tensor("xa", [128, NMT * MTW], bf16, kind="ExternalInput")
    wa = nc.dram_tensor("wa", [128, NK * D], bf16, kind="ExternalInput")
    out = nc.dram_tensor("out", [128, NMT * D], bf16, kind="ExternalOutput")

    with tile.TileContext(nc) as tc:
        with (
            tc.tile_pool(name="w", bufs=1) as wpool,
            tc.tile_pool(name="x", bufs=1) as xpool,
            tc.tile_pool(name="o", bufs=2) as opool,
            tc.tile_pool(name="ps", bufs=4, space="PSUM") as pspool,
        ):
            # Warm the PE (HAM clock gate) with dummy matmuls while the
            # first DMAs stream in. Memset on DVE so it isn't gated on the
            # slower GpSimd preamble; dummy MMs land in a psum pool slot
            # that gets recycled (start=True clears it before any real use).
            scratch = wpool.tile([128, 512], bf16, tag="scratch")
            nc.vector.memset(scratch[:], 0.0)
            dps = pspool.tile([128, 2 * 512], f32, tag="ps")
            for i in range(14):
                nc.tensor.matmul(
                    dps[:, 0:512], scratch[:, 0:128], scratch[:],
                    start=(i == 0), stop=(i == 13),
                )

            wts = []
            xtiles = {}

            def dma_w(j):
                # 1 MB tiles (8 KB per-partition lines) keep the DMA near
                # line rate; small chunks with 2 KB lines run at ~half BW
                wt = wpool.tile([128, 4 * D], bf16, tag=f"w{j}")
                nc.sync.dma_start(wt[:], wa[:, j * 4 * D : (j + 1) * 4 * D])
                wts.append(wt)

            def dma_x(g):
                mts = X_PLAN[g]
                t = xpool.tile([128, len(mts) * MTW], bf16, tag=f"x{g}")
                nc.sync.dma_start(
                    t[:], xa[:, mts[0] * MTW : (mts[-1] + 1) * MTW]
                )
                for i, mt in enumerate(mts):
                    xtiles[mt] = (t, i * MTW)

            # issue order = consumption order, alternating the two gating
            # streams: W half, x block 0, W half, then the rest of x
            dma_w(0)
            dma_x(0)
            dma_w(1)
            for g in range(1, len(X_PLAN)):
                dma_x(g)

            ot = None
            for mt in range(NMT):
                xt, xof = xtiles[mt]
                ps = pspool.tile([128, 2 * 512], f32, tag="ps")
                last = mt == NMT - 1
                if not last:
                    for k in range(NK):
                        wt = wts[k // 4]
                        wof = (k % 4) * D
                        lhsT = xt[:, xof + k * 128 : xof + (k + 1) * 128]
                        nc.tensor.matmul(
                            ps[:, 0:512], lhsT, wt[:, wof : wof + 512],
                            start=(k == 0), stop=(k == NK - 1),
                        )
                        nc.tensor.matmul(
                            ps[:, 512:1024], lhsT, wt[:, wof + 512 : wof + D],
                            start=(k == 0), stop=(k == NK - 1),
                        )
                if mt % 2 == 0:
                    ot = opool.tile([128, 2 * D], bf16, tag="ot")
                oof = (mt % 2) * D
                if last:
                    # final m-tile: run each n-half's k-chain to completion,
                    # then immediately add + store that half so it pipelines
                    # with the other half's matmuls (shrinks the drain tail)
                    for h in range(2):
                        for k in range(NK):
                            wt = wts[k // 4]
                            wof = (k % 4) * D + h * 512
                            lhsT = xt[:, xof + k * 128 : xof + (k + 1) * 128]
                            nc.tensor.matmul(
                                ps[:, h * 512 : (h + 1) * 512],
                                lhsT,
                                wt[:, wof : wof + 512],
                                start=(k == 0), stop=(k == NK - 1),
                            )
                        nc.vector.tensor_add(
                            ot[:, oof + h * 512 : oof + (h + 1) * 512],
                            ps[:, h * 512 : (h + 1) * 512],
                            xt[:, xof + D + h * 512 : xof + D + (h + 1) * 512],
                        )
                        nc.scalar.dma_start(
                            out[:, mt * D + h * 512 : mt * D + (h + 1) * 512],
                            ot[:, oof + h * 512 : oof + (h + 1) * 512],
                        )
                else:
                    nc.vector.tensor_add(
                        ot[:, oof : oof + D],
                        ps[:],
                        xt[:, xof + D : xof + 2 * D],
                    )
                    if mt == NMT - 2:
                        nc.scalar.dma_start(
                            out[:, mt * D : (mt + 1) * D], ot[:, oof : oof + D]
                        )
                    elif mt % 2 == 1:
                        nc.scalar.dma_start(
                            out[:, (mt - 1) * D : (mt + 1) * D], ot[:]
                        )

    nc.compile()
    return nc


def _prepare(x, W, A_table, B_table, domain_id):
    import ml_dtypes

    bf16 = np.dtype(ml_dtypes.bfloat16)
    x = np.asarray(x, dtype=np.float32)
    W = np.asarray(W, dtype=np.float32)
    A = np.asarray(A_table, dtype=np.float64)
    Bt = np.asarray(B_table, dtype=np.float64)
    dom = np.asarray(domain_id).astype(np.int64)

    sA = A.reshape(ND, R, D).sum(axis=2)                        # [ND, R]
    L = np.einsum("dr,dro->do", sA, Bt.reshape(ND, R, D))       # [ND, D]
    Lg = L.astype(np.float32)[dom].astype(bf16)                 # [B, D]

    # W.T chunk-major: wa[p, k*D + n] = W.T[k*128+p, n]
    wa = np.ascontiguousarray(
        W.T.astype(bf16).reshape(NK, 128, D).transpose(1, 0, 2)
    ).reshape(128, NK * D)

    in_maps = []
    for c in range(N_CORES):
        sl = slice(c * BS, (c + 1) * BS)
        xc = x[sl].astype(bf16)                                 # [2048, 1024]
        # xpart[p, mt, k*128+j] = xc[mt*128+j, k*128+p]
        xpart = xc.reshape(NMT, 128, NK, 128).transpose(3, 0, 2, 1)
        lgpart = Lg[sl].reshape(NMT, 128, D).transpose(1, 0, 2)  # [p, mt, n]
        xaug = np.empty((128, NMT, MTW), dtype=bf16)
        xaug[:, :, 0:D] = xpart.reshape(128, NMT, D)
        xaug[:, :, D:MTW] = lgpart
        in_maps.append({"xa": xaug.reshape(128, NMT * MTW), "wa": wa})
    return in_maps


def kernel(x, W, A_table, B_table, domain_id, _trace=False):
    in_maps = _prepare(x, W, A_table, B_table, domain_id)
    nc = _build()
    res = bass_utils.run_bass_kernel_spmd(
        nc, in_maps, core_ids=list(range(N_CORES)), trace=_trace
    )
    outs = []
    for c in range(N_CORES):
        oc = res.results[c]["out"]                              # [128, NMT*D] bf16
        outs.append(
            oc.reshape(128, NMT, D)
            .transpose(1, 0, 2)
            .reshape(BS, D)
            .astype(np.float32)
        )
    out = np.concatenate(outs, axis=0)
    if _trace:
        kernel.last_results = res
    return out
